# revision 7
# baseline (speedup 1.0000x reference)
"""Trainium2 Bass kernel for nn_AttnDecoderLSTM (B=4096, S=512, H=O=128).

Data-parallel over 8 NeuronCores: each core owns 512 batch rows.
All on-chip compute runs in transposed (feature-on-partition) layout;
the host pre-transposes the small per-core activations and weights and
un-transposes the small outputs.
"""

import os
import sys

sys.path.insert(0, "/opt/trn_rl_repo")

import numpy as np

import concourse.bass as bass  # noqa: F401  (AP types)
import concourse.tile as tile
from concourse import bacc, mybir
from concourse.bass_utils import run_bass_kernel_spmd
from concourse.masks import make_identity

F32 = mybir.dt.float32
AF = mybir.ActivationFunctionType

B, S, H, O = 4096, 512, 128, 128
N_CORES = 8
BC = B // N_CORES  # batch rows per core (512)
P = 128  # tile size along batch
NCH = S // 128  # s-chunks (4)
NB = int(os.environ.get("K_NB", "2"))  # batch rows per encoder DMA
MATVEC_DT = os.environ.get("K_MVDT", "bf16")  # "f32" | "f32r" | "bf16"
NW = int(os.environ.get("K_NW", "16"))  # HAM-warmer cadence in batch rows (0 = off)
EPOOL = int(os.environ.get("K_EPOOL", "6"))  # encoder tile double-buffer depth
BF16 = mybir.dt.bfloat16

_NC_CACHE: dict = {}


def build_nc(bc: int = BC, nb: int = NB, mv_dt: str = MATVEC_DT, nw: int = NW):
    """Build the per-core Bass program (same program on all cores)."""
    nc = bacc.Bacc("TRN2", target_bir_lowering=False)

    # ---- DRAM I/O (per-core shard shapes) ----
    xT_d = nc.dram_tensor("xT", [O, bc], F32, kind="ExternalInput")
    hT_d = nc.dram_tensor("hT", [H, bc], F32, kind="ExternalInput")
    cT_d = nc.dram_tensor("cT", [H, bc], F32, kind="ExternalInput")
    enc_d = nc.dram_tensor("enc", [bc, S, H], F32, kind="ExternalInput")

    attn_Wt_x_d = nc.dram_tensor("attn_Wt_x", [O, S], F32, kind="ExternalInput")
    attn_Wt_h_d = nc.dram_tensor("attn_Wt_h", [H, S], F32, kind="ExternalInput")
    attn_b_d = nc.dram_tensor("attn_b", [1, S], F32, kind="ExternalInput")
    comb_Wt_x_d = nc.dram_tensor("comb_Wt_x", [O, O], F32, kind="ExternalInput")
    comb_Wt_c_d = nc.dram_tensor("comb_Wt_c", [H, O], F32, kind="ExternalInput")
    comb_b_d = nc.dram_tensor("comb_b", [O, 1], F32, kind="ExternalInput")
    W_ih_t_d = nc.dram_tensor("W_ih_t", [O, 4 * H], F32, kind="ExternalInput")
    W_hh_t_d = nc.dram_tensor("W_hh_t", [H, 4 * H], F32, kind="ExternalInput")
    gbias_d = nc.dram_tensor("gbias", [H, 4], F32, kind="ExternalInput")
    out_Wt_d = nc.dram_tensor("out_Wt", [H, O], F32, kind="ExternalInput")
    out_b_d = nc.dram_tensor("out_b", [O, 1], F32, kind="ExternalInput")

    outT_d = nc.dram_tensor("outT", [O, bc], F32, kind="ExternalOutput")
    h1T_d = nc.dram_tensor("h1T", [H, bc], F32, kind="ExternalOutput")
    c1T_d = nc.dram_tensor("c1T", [H, bc], F32, kind="ExternalOutput")
    attn_d = nc.dram_tensor("attn", [bc, S], F32, kind="ExternalOutput")
    hamd_d = (
        nc.dram_tensor("hamd", [128, 4], F32, kind="ExternalOutput") if nw else None
    )

    ntiles = bc // P

    with tile.TileContext(nc) as tc:
        with (
            tc.tile_pool(name="wpool", bufs=1) as wp,
            tc.tile_pool(name="epool", bufs=EPOOL) as ep,
            tc.tile_pool(name="xpool", bufs=2) as xp,
            tc.tile_pool(name="apool", bufs=2) as ap,
            tc.tile_pool(name="spool", bufs=2) as sp,
            tc.tile_pool(name="ps_logit", bufs=2, space="PSUM") as ps_logit,
            tc.tile_pool(name="ps_ctx", bufs=2, space="PSUM") as ps_ctx,
            tc.tile_pool(name="ps_small", bufs=3, space="PSUM") as ps_small,
            tc.tile_pool(name="ps_ham", bufs=1, space="PSUM") as ps_ham,
        ):
            # ---- constants / weights (loaded once) ----
            ident = wp.tile([128, 128], F32)
            make_identity(nc, ident[:])
            ones_row = wp.tile([1, P], F32)
            nc.gpsimd.memset(ones_row[:], 1.0)
            if nw:
                zz = wp.tile([128, 512], BF16)
                nc.gpsimd.memset(zz[:], 0.0)
                ham_ps = ps_ham.tile([128, 512], F32)

            def wload(dram, shape):
                t = wp.tile(shape, F32, tag=dram.name)
                nc.scalar.dma_start(t[:], dram[:])
                return t

            attn_Wt_x = wload(attn_Wt_x_d, [O, S])
            attn_Wt_h = wload(attn_Wt_h_d, [H, S])
            attn_b = wload(attn_b_d, [1, S])
            comb_Wt_x = wload(comb_Wt_x_d, [O, O])
            comb_Wt_c = wload(comb_Wt_c_d, [H, O])
            comb_b = wload(comb_b_d, [O, 1])
            W_ih_t = wload(W_ih_t_d, [O, 4 * H])
            W_hh_t = wload(W_hh_t_d, [H, 4 * H])
            gbias = wload(gbias_d, [H, 4])
            out_Wt = wload(out_Wt_d, [H, O])
            out_b = wload(out_b_d, [O, 1])

            at_dt = BF16 if mv_dt == "bf16" else F32
            e_dt = BF16 if mv_dt == "bf16" else F32

            xT_all = wp.tile([O, bc], F32)
            hT_all = wp.tile([H, bc], F32)
            cT_all = wp.tile([H, bc], F32)
            nc.scalar.dma_start(xT_all[:], xT_d[:])
            nc.scalar.dma_start(hT_all[:], hT_d[:])
            nc.scalar.dma_start(cT_all[:], cT_d[:])
            h1_all = wp.tile([H, bc], F32)
            c1_all = wp.tile([H, bc], F32)
            out_all = wp.tile([O, bc], F32)

            def stage_a(t):
                """Inputs, attention logits, softmax, transposed attn columns."""
                tsl = slice(t * P, (t + 1) * P)
                xT = xT_all[:, tsl]
                hT = hT_all[:, tsl]
                cT = cT_all[:, tsl]

                logits = ps_logit.tile([P, S], F32)
                nc.tensor.matmul(logits[:], xT, attn_Wt_x[:], start=True, stop=False)
                nc.tensor.matmul(logits[:], hT, attn_Wt_h[:], start=False, stop=False)
                nc.tensor.matmul(
                    logits[:], ones_row[:], attn_b[:], start=False, stop=True
                )

                negmax = ap.tile([P, 1], F32, tag="negmax")
                nc.vector.reduce_max(
                    negmax[:], logits[:], axis=mybir.AxisListType.X, negate=True
                )
                A = ap.tile([P, S], F32, tag="A")
                esum = ap.tile([P, 1], F32, tag="esum")
                nc.scalar.activation(
                    A[:], logits[:], AF.Exp, bias=negmax[:], accum_out=esum[:]
                )
                rs = ap.tile([P, 1], F32, tag="rs")
                nc.vector.reciprocal(rs[:], esum[:])
                nc.vector.tensor_scalar_mul(A[:], A[:], rs[:])
                nc.scalar.dma_start(attn_d[tsl, :], A[:])

                ATs = []
                for c in range(NCH):
                    ps = ps_small.tile([128, P], F32, tag="ps_s")
                    nc.tensor.transpose(ps[:], A[:, c::NCH], ident[:])
                    ATc = ap.tile([128, P], at_dt, tag=f"AT{c}")
                    nc.scalar.copy(ATc[:], ps[:])
                    ATs.append(ATc)
                return dict(xT=xT, hT=hT, cT=cT, ATs=ATs)

            def stage_bcde(t, st):
                tsl = slice(t * P, (t + 1) * P)
                xT, hT, cT, ATs = st["xT"], st["hT"], st["cT"], st["ATs"]

                # ---- stage B: ctx^T accumulation (the heavy stream) ----
                ctxT_ps = ps_ctx.tile([H, P], F32)
                for bb in range(0, P, nb):
                    if nw and bb % nw == 0:
                        nc.tensor.matmul(
                            ham_ps[:], zz[:, :128], zz[:], start=True, stop=True
                        )
                    et = ep.tile([128, nb, NCH, H], e_dt, tag="E")
                    esrc = enc_d[t * P + bb : t * P + bb + nb, :, :].rearrange(
                        "bb (p j) h -> p bb j h", j=NCH
                    )
                    if mv_dt == "bf16":
                        nc.gpsimd.dma_start(et[:], esrc)
                    else:
                        nc.sync.dma_start(et[:], esrc)
                    for j in range(nb):
                        col = slice(bb + j, bb + j + 1)
                        for c in range(NCH):
                            nc.tensor.matmul(
                                ctxT_ps[:, col],
                                et[:, j, c, :],
                                ATs[c][:, col],
                                start=(c == 0),
                                stop=(c == NCH - 1),
                            )
                ctxT = sp.tile([H, P], F32, tag="ctxT")
                nc.scalar.copy(ctxT[:], ctxT_ps[:])

                # ---- stage C: comb = relu([x, ctx] @ comb_W.T + comb_b) ----
                comb_ps = ps_small.tile([O, P], F32, tag="ps_s")
                nc.tensor.matmul(comb_ps[:], comb_Wt_x[:], xT, start=True, stop=False)
                nc.tensor.matmul(
                    comb_ps[:], comb_Wt_c[:], ctxT[:], start=False, stop=True
                )
                combT = sp.tile([O, P], F32, tag="combT")
                nc.scalar.activation(combT[:], comb_ps[:], AF.Relu, bias=comb_b[:])

                # ---- stage D: LSTM cell (gates: 0=i, 1=f, 2=g, 3=o) ----
                gact = []
                for g in range(4):
                    gsl = slice(g * H, (g + 1) * H)
                    gps = ps_small.tile([H, P], F32, tag="ps_s")
                    nc.tensor.matmul(
                        gps[:], W_ih_t[:, gsl], combT[:], start=True, stop=False
                    )
                    nc.tensor.matmul(
                        gps[:], W_hh_t[:, gsl], hT, start=False, stop=True
                    )
                    gs = sp.tile([H, P], F32, tag=f"g{g}")
                    if g == 2:
                        nc.scalar.activation(gs[:], gps[:], AF.Tanh, bias=gbias[:, 2:3])
                    else:
                        # sigmoid(x+b) = 0.5*tanh(0.5x+0.5b)+0.5
                        nc.scalar.activation(
                            gs[:], gps[:], AF.Tanh, bias=gbias[:, g : g + 1], scale=0.5
                        )
                        nc.vector.tensor_scalar(
                            gs[:], gs[:], 0.5, 0.5,
                            op0=mybir.AluOpType.mult, op1=mybir.AluOpType.add,
                        )
                    gact.append(gs)

                i_g, f_g, g_g, o_g = gact
                t1 = sp.tile([H, P], F32, tag="t1")
                nc.vector.tensor_mul(t1[:], f_g[:], cT)
                t2 = sp.tile([H, P], F32, tag="t2")
                nc.vector.tensor_mul(t2[:], i_g[:], g_g[:])
                c1T = c1_all[:, tsl]
                nc.vector.tensor_add(c1T, t1[:], t2[:])

                tc1 = sp.tile([H, P], F32, tag="tc1")
                nc.scalar.activation(tc1[:], c1T, AF.Tanh)
                h1T = h1_all[:, tsl]
                nc.vector.tensor_mul(h1T, o_g[:], tc1[:])

                # ---- stage E: out = h1 @ out_W.T + out_b ----
                out_ps = ps_small.tile([O, P], F32, tag="ps_s")
                nc.tensor.matmul(out_ps[:], out_Wt[:], h1T, start=True, stop=True)
                nc.scalar.activation(
                    out_all[:, tsl], out_ps[:], AF.Identity, bias=out_b[:]
                )

            # Software pipeline: stage A of tile t+1 is emitted before the
            # heavy stream of tile t so the PE has the next tile's attention
            # columns ready before its matvecs begin.
            st = stage_a(0)
            for t in range(ntiles):
                nxt = stage_a(t + 1) if t + 1 < ntiles else None
                stage_bcde(t, st)
                st = nxt

            nc.scalar.dma_start(h1T_d[:], h1_all[:])
            nc.scalar.dma_start(c1T_d[:], c1_all[:])
            nc.scalar.dma_start(outT_d[:], out_all[:])

            if nw:
                hs = wp.tile([128, 4], F32)
                nc.vector.tensor_copy(hs[:], ham_ps[:, :4])
                nc.scalar.dma_start(hamd_d[:], hs[:])

    nc.compile()
    return nc


def _prep_in_maps(
    input, h, c, encoder_outputs, attn_W, attn_b, comb_W, comb_b,
    W_ih, b_ih, W_hh, b_hh, out_W, out_b, bc: int = BC, n_cores: int = N_CORES,
):
    f32 = np.float32
    x = np.ascontiguousarray(input[:, 0, :], dtype=f32)  # [B, O]
    h0 = np.ascontiguousarray(h[0], dtype=f32)  # [B, H]
    c0 = np.ascontiguousarray(c[0], dtype=f32)  # [B, H]
    enc = np.ascontiguousarray(encoder_outputs, dtype=f32)

    gb = (np.asarray(b_ih, f32) + np.asarray(b_hh, f32)).reshape(4, H)
    gbias = gb.copy()
    for g in (0, 1, 3):
        gbias[g] *= 0.5  # folded into the 0.5*tanh(0.5x+0.5b)+0.5 sigmoid
    weights = {
        "attn_Wt_x": np.ascontiguousarray(np.asarray(attn_W, f32).T[:O], f32),
        "attn_Wt_h": np.ascontiguousarray(np.asarray(attn_W, f32).T[O:], f32),
        "attn_b": np.ascontiguousarray(np.asarray(attn_b, f32)[None, :], f32),
        "comb_Wt_x": np.ascontiguousarray(np.asarray(comb_W, f32).T[:O], f32),
        "comb_Wt_c": np.ascontiguousarray(np.asarray(comb_W, f32).T[O:], f32),
        "comb_b": np.ascontiguousarray(np.asarray(comb_b, f32)[:, None], f32),
        "W_ih_t": np.ascontiguousarray(np.asarray(W_ih, f32).T, f32),
        "W_hh_t": np.ascontiguousarray(np.asarray(W_hh, f32).T, f32),
        "gbias": np.ascontiguousarray(gbias.T, f32),  # [H, 4]
        "out_Wt": np.ascontiguousarray(np.asarray(out_W, f32).T, f32),
        "out_b": np.ascontiguousarray(np.asarray(out_b, f32)[:, None], f32),
    }
    in_maps = []
    for cidx in range(n_cores):
        rows = slice(cidx * bc, (cidx + 1) * bc)
        m = dict(weights)
        m["xT"] = np.ascontiguousarray(x[rows].T, f32)
        m["hT"] = np.ascontiguousarray(h0[rows].T, f32)
        m["cT"] = np.ascontiguousarray(c0[rows].T, f32)
        m["enc"] = enc[rows]
        in_maps.append(m)
    return in_maps


def _gather(results, bc: int = BC):
    out = np.concatenate([r["outT"].T for r in results], axis=0)
    h1 = np.concatenate([r["h1T"].T for r in results], axis=0)
    c1 = np.concatenate([r["c1T"].T for r in results], axis=0)
    attn = np.concatenate([r["attn"] for r in results], axis=0)
    return (
        np.ascontiguousarray(out, np.float32),
        np.ascontiguousarray(h1, np.float32)[None],
        np.ascontiguousarray(c1, np.float32)[None],
        np.ascontiguousarray(attn, np.float32),
    )


def _install_ntff_hook():
    """The image's antenv lacks axon_hooks; provide it and register the
    ctypes NTFF profiling hook so trace=True yields exec_time_ns."""
    import sys
    import types

    if "antenv.axon_hooks" in sys.modules:
        return
    import antenv

    mod = types.ModuleType("antenv.axon_hooks")
    _holder = {"hook": None}
    mod.set_axon_ntff_profile_hook = lambda h: _holder.__setitem__("hook", h)
    mod.get_axon_ntff_profile_hook = lambda: _holder["hook"]
    sys.modules["antenv.axon_hooks"] = mod
    antenv.axon_hooks = mod
    try:
        from trn_agent_boot.trn_boot import _ntff_profile_via_ctypes

        hook = _ntff_profile_via_ctypes("/opt/axon/libaxon_pjrt.so")
        if hook is not None:
            mod.set_axon_ntff_profile_hook(hook)
    except Exception as e:  # degrade: tracing skipped, run still works
        print(f"ntff hook install failed: {e}")


def run(inputs: dict, trace: bool = False, bc: int = BC, n_cores: int = N_CORES):
    if trace:
        _install_ntff_hook()
    key = (bc, n_cores)
    if key not in _NC_CACHE:
        _NC_CACHE[key] = build_nc(bc=bc)
    nc = _NC_CACHE[key]
    in_maps = _prep_in_maps(**inputs, bc=bc, n_cores=n_cores)
    res = run_bass_kernel_spmd(
        nc, in_maps, list(range(n_cores)), trace=trace,
        tmpdir=os.environ.get("BASS_TRACE_DIR"),
    )
    return _gather(res.results, bc=bc), res


def kernel(**inputs):
    outputs, _ = run(inputs)
    return outputs


# revision 8
# speedup vs baseline: 1.0026x; 1.0026x over previous
"""Trainium2 Bass kernel for nn_AttnDecoderLSTM (B=4096, S=512, H=O=128).

Data-parallel over 8 NeuronCores: each core owns 512 batch rows.
All on-chip compute runs in transposed (feature-on-partition) layout;
the host pre-transposes the small per-core activations and weights and
un-transposes the small outputs.
"""

import os
import sys

sys.path.insert(0, "/opt/trn_rl_repo")

import numpy as np

import concourse.bass as bass  # noqa: F401  (AP types)
import concourse.tile as tile
from concourse import bacc, mybir
from concourse.bass_utils import run_bass_kernel_spmd
from concourse.masks import make_identity

F32 = mybir.dt.float32
AF = mybir.ActivationFunctionType

B, S, H, O = 4096, 512, 128, 128
N_CORES = 8
BC = B // N_CORES  # batch rows per core (512)
P = 128  # tile size along batch
NCH = S // 128  # s-chunks (4)
NB = int(os.environ.get("K_NB", "4"))  # batch rows per encoder DMA
MATVEC_DT = os.environ.get("K_MVDT", "bf16")  # "f32" | "f32r" | "bf16"
NW = int(os.environ.get("K_NW", "0"))  # HAM-warmer cadence in batch rows (0 = off)
EPOOL = int(os.environ.get("K_EPOOL", "12"))  # encoder tile double-buffer depth
BF16 = mybir.dt.bfloat16

_NC_CACHE: dict = {}


def build_nc(bc: int = BC, nb: int = NB, mv_dt: str = MATVEC_DT, nw: int = NW):
    """Build the per-core Bass program (same program on all cores)."""
    nc = bacc.Bacc("TRN2", target_bir_lowering=False)

    # ---- DRAM I/O (per-core shard shapes) ----
    xT_d = nc.dram_tensor("xT", [O, bc], F32, kind="ExternalInput")
    hT_d = nc.dram_tensor("hT", [H, bc], F32, kind="ExternalInput")
    cT_d = nc.dram_tensor("cT", [H, bc], F32, kind="ExternalInput")
    enc_d = nc.dram_tensor("enc", [bc, S, H], F32, kind="ExternalInput")

    attn_Wt_x_d = nc.dram_tensor("attn_Wt_x", [O, S], F32, kind="ExternalInput")
    attn_Wt_h_d = nc.dram_tensor("attn_Wt_h", [H, S], F32, kind="ExternalInput")
    attn_b_d = nc.dram_tensor("attn_b", [1, S], F32, kind="ExternalInput")
    comb_Wt_x_d = nc.dram_tensor("comb_Wt_x", [O, O], F32, kind="ExternalInput")
    comb_Wt_c_d = nc.dram_tensor("comb_Wt_c", [H, O], F32, kind="ExternalInput")
    comb_b_d = nc.dram_tensor("comb_b", [O, 1], F32, kind="ExternalInput")
    W_ih_t_d = nc.dram_tensor("W_ih_t", [O, 4 * H], F32, kind="ExternalInput")
    W_hh_t_d = nc.dram_tensor("W_hh_t", [H, 4 * H], F32, kind="ExternalInput")
    gbias_d = nc.dram_tensor("gbias", [H, 4], F32, kind="ExternalInput")
    out_Wt_d = nc.dram_tensor("out_Wt", [H, O], F32, kind="ExternalInput")
    out_b_d = nc.dram_tensor("out_b", [O, 1], F32, kind="ExternalInput")

    outT_d = nc.dram_tensor("outT", [O, bc], F32, kind="ExternalOutput")
    h1T_d = nc.dram_tensor("h1T", [H, bc], F32, kind="ExternalOutput")
    c1T_d = nc.dram_tensor("c1T", [H, bc], F32, kind="ExternalOutput")
    attn_d = nc.dram_tensor("attn", [bc, S], F32, kind="ExternalOutput")
    hamd_d = (
        nc.dram_tensor("hamd", [128, 4], F32, kind="ExternalOutput") if nw else None
    )

    ntiles = bc // P

    with tile.TileContext(nc) as tc:
        with (
            tc.tile_pool(name="wpool", bufs=1) as wp,
            tc.tile_pool(name="epool", bufs=EPOOL) as ep,
            tc.tile_pool(name="xpool", bufs=2) as xp,
            tc.tile_pool(name="apool", bufs=2) as ap,
            tc.tile_pool(name="spool", bufs=2) as sp,
            tc.tile_pool(name="ps_logit", bufs=2, space="PSUM") as ps_logit,
            tc.tile_pool(name="ps_ctx", bufs=2, space="PSUM") as ps_ctx,
            tc.tile_pool(name="ps_small", bufs=3, space="PSUM") as ps_small,
            tc.tile_pool(name="ps_ham", bufs=1, space="PSUM") as ps_ham,
        ):
            # ---- constants / weights (loaded once) ----
            ident = wp.tile([128, 128], F32)
            make_identity(nc, ident[:])
            ones_row = wp.tile([1, P], F32)
            nc.gpsimd.memset(ones_row[:], 1.0)
            if nw:
                zz = wp.tile([128, 512], BF16)
                nc.gpsimd.memset(zz[:], 0.0)
                ham_ps = ps_ham.tile([128, 512], F32)

            def wload(dram, shape):
                t = wp.tile(shape, F32, tag=dram.name)
                nc.scalar.dma_start(t[:], dram[:])
                return t

            attn_Wt_x = wload(attn_Wt_x_d, [O, S])
            attn_Wt_h = wload(attn_Wt_h_d, [H, S])
            attn_b = wload(attn_b_d, [1, S])
            comb_Wt_x = wload(comb_Wt_x_d, [O, O])
            comb_Wt_c = wload(comb_Wt_c_d, [H, O])
            comb_b = wload(comb_b_d, [O, 1])
            W_ih_t = wload(W_ih_t_d, [O, 4 * H])
            W_hh_t = wload(W_hh_t_d, [H, 4 * H])
            gbias = wload(gbias_d, [H, 4])
            out_Wt = wload(out_Wt_d, [H, O])
            out_b = wload(out_b_d, [O, 1])

            at_dt = BF16 if mv_dt == "bf16" else F32
            e_dt = BF16 if mv_dt == "bf16" else F32

            xT_all = wp.tile([O, bc], F32)
            hT_all = wp.tile([H, bc], F32)
            cT_all = wp.tile([H, bc], F32)
            nc.scalar.dma_start(xT_all[:], xT_d[:])
            nc.scalar.dma_start(hT_all[:], hT_d[:])
            nc.scalar.dma_start(cT_all[:], cT_d[:])
            h1_all = wp.tile([H, bc], F32)
            c1_all = wp.tile([H, bc], F32)
            out_all = wp.tile([O, bc], F32)

            def stage_a(t):
                """Inputs, attention logits, softmax, transposed attn columns."""
                tsl = slice(t * P, (t + 1) * P)
                xT = xT_all[:, tsl]
                hT = hT_all[:, tsl]
                cT = cT_all[:, tsl]

                logits = ps_logit.tile([P, S], F32)
                nc.tensor.matmul(logits[:], xT, attn_Wt_x[:], start=True, stop=False)
                nc.tensor.matmul(logits[:], hT, attn_Wt_h[:], start=False, stop=False)
                nc.tensor.matmul(
                    logits[:], ones_row[:], attn_b[:], start=False, stop=True
                )

                negmax = ap.tile([P, 1], F32, tag="negmax")
                nc.vector.reduce_max(
                    negmax[:], logits[:], axis=mybir.AxisListType.X, negate=True
                )
                A = ap.tile([P, S], F32, tag="A")
                esum = ap.tile([P, 1], F32, tag="esum")
                nc.scalar.activation(
                    A[:], logits[:], AF.Exp, bias=negmax[:], accum_out=esum[:]
                )
                rs = ap.tile([P, 1], F32, tag="rs")
                nc.vector.reciprocal(rs[:], esum[:])
                nc.vector.tensor_scalar_mul(A[:], A[:], rs[:])
                nc.scalar.dma_start(attn_d[tsl, :], A[:])

                ATs = []
                for c in range(NCH):
                    ps = ps_small.tile([128, P], F32, tag="ps_s")
                    nc.tensor.transpose(ps[:], A[:, c::NCH], ident[:])
                    ATc = ap.tile([128, P], at_dt, tag=f"AT{c}")
                    nc.scalar.copy(ATc[:], ps[:])
                    ATs.append(ATc)
                return dict(xT=xT, hT=hT, cT=cT, ATs=ATs)

            def stage_bcde(t, st):
                tsl = slice(t * P, (t + 1) * P)
                xT, hT, cT, ATs = st["xT"], st["hT"], st["cT"], st["ATs"]

                # ---- stage B: ctx^T accumulation (the heavy stream) ----
                ctxT_ps = ps_ctx.tile([H, P], F32)
                for bb in range(0, P, nb):
                    if nw and bb % nw == 0:
                        nc.tensor.matmul(
                            ham_ps[:], zz[:, :128], zz[:], start=True, stop=True
                        )
                    et = ep.tile([128, nb, NCH, H], e_dt, tag="E")
                    esrc = enc_d[t * P + bb : t * P + bb + nb, :, :].rearrange(
                        "bb (p j) h -> p bb j h", j=NCH
                    )
                    if mv_dt == "bf16":
                        nc.gpsimd.dma_start(et[:], esrc)
                    else:
                        nc.sync.dma_start(et[:], esrc)
                    for j in range(nb):
                        col = slice(bb + j, bb + j + 1)
                        for c in range(NCH):
                            nc.tensor.matmul(
                                ctxT_ps[:, col],
                                et[:, j, c, :],
                                ATs[c][:, col],
                                start=(c == 0),
                                stop=(c == NCH - 1),
                            )
                ctxT = sp.tile([H, P], F32, tag="ctxT")
                nc.scalar.copy(ctxT[:], ctxT_ps[:])

                # ---- stage C: comb = relu([x, ctx] @ comb_W.T + comb_b) ----
                comb_ps = ps_small.tile([O, P], F32, tag="ps_s")
                nc.tensor.matmul(comb_ps[:], comb_Wt_x[:], xT, start=True, stop=False)
                nc.tensor.matmul(
                    comb_ps[:], comb_Wt_c[:], ctxT[:], start=False, stop=True
                )
                combT = sp.tile([O, P], F32, tag="combT")
                nc.scalar.activation(combT[:], comb_ps[:], AF.Relu, bias=comb_b[:])

                # ---- stage D: LSTM cell (gates: 0=i, 1=f, 2=g, 3=o) ----
                gact = []
                for g in range(4):
                    gsl = slice(g * H, (g + 1) * H)
                    gps = ps_small.tile([H, P], F32, tag="ps_s")
                    nc.tensor.matmul(
                        gps[:], W_ih_t[:, gsl], combT[:], start=True, stop=False
                    )
                    nc.tensor.matmul(
                        gps[:], W_hh_t[:, gsl], hT, start=False, stop=True
                    )
                    gs = sp.tile([H, P], F32, tag=f"g{g}")
                    if g == 2:
                        nc.scalar.activation(gs[:], gps[:], AF.Tanh, bias=gbias[:, 2:3])
                    else:
                        # sigmoid(x+b) = 0.5*tanh(0.5x+0.5b)+0.5
                        nc.scalar.activation(
                            gs[:], gps[:], AF.Tanh, bias=gbias[:, g : g + 1], scale=0.5
                        )
                        nc.vector.tensor_scalar(
                            gs[:], gs[:], 0.5, 0.5,
                            op0=mybir.AluOpType.mult, op1=mybir.AluOpType.add,
                        )
                    gact.append(gs)

                i_g, f_g, g_g, o_g = gact
                t1 = sp.tile([H, P], F32, tag="t1")
                nc.vector.tensor_mul(t1[:], f_g[:], cT)
                t2 = sp.tile([H, P], F32, tag="t2")
                nc.vector.tensor_mul(t2[:], i_g[:], g_g[:])
                c1T = c1_all[:, tsl]
                nc.vector.tensor_add(c1T, t1[:], t2[:])

                tc1 = sp.tile([H, P], F32, tag="tc1")
                nc.scalar.activation(tc1[:], c1T, AF.Tanh)
                h1T = h1_all[:, tsl]
                nc.vector.tensor_mul(h1T, o_g[:], tc1[:])

                # ---- stage E: out = h1 @ out_W.T + out_b ----
                out_ps = ps_small.tile([O, P], F32, tag="ps_s")
                nc.tensor.matmul(out_ps[:], out_Wt[:], h1T, start=True, stop=True)
                nc.scalar.activation(
                    out_all[:, tsl], out_ps[:], AF.Identity, bias=out_b[:]
                )

            # Software pipeline: stage A of tile t+1 is emitted before the
            # heavy stream of tile t so the PE has the next tile's attention
            # columns ready before its matvecs begin.
            st = stage_a(0)
            for t in range(ntiles):
                nxt = stage_a(t + 1) if t + 1 < ntiles else None
                stage_bcde(t, st)
                st = nxt

            nc.scalar.dma_start(h1T_d[:], h1_all[:])
            nc.scalar.dma_start(c1T_d[:], c1_all[:])
            nc.scalar.dma_start(outT_d[:], out_all[:])

            if nw:
                hs = wp.tile([128, 4], F32)
                nc.vector.tensor_copy(hs[:], ham_ps[:, :4])
                nc.scalar.dma_start(hamd_d[:], hs[:])

    nc.compile()
    return nc


def _prep_in_maps(
    input, h, c, encoder_outputs, attn_W, attn_b, comb_W, comb_b,
    W_ih, b_ih, W_hh, b_hh, out_W, out_b, bc: int = BC, n_cores: int = N_CORES,
):
    f32 = np.float32
    x = np.ascontiguousarray(input[:, 0, :], dtype=f32)  # [B, O]
    h0 = np.ascontiguousarray(h[0], dtype=f32)  # [B, H]
    c0 = np.ascontiguousarray(c[0], dtype=f32)  # [B, H]
    enc = np.ascontiguousarray(encoder_outputs, dtype=f32)

    gb = (np.asarray(b_ih, f32) + np.asarray(b_hh, f32)).reshape(4, H)
    gbias = gb.copy()
    for g in (0, 1, 3):
        gbias[g] *= 0.5  # folded into the 0.5*tanh(0.5x+0.5b)+0.5 sigmoid
    weights = {
        "attn_Wt_x": np.ascontiguousarray(np.asarray(attn_W, f32).T[:O], f32),
        "attn_Wt_h": np.ascontiguousarray(np.asarray(attn_W, f32).T[O:], f32),
        "attn_b": np.ascontiguousarray(np.asarray(attn_b, f32)[None, :], f32),
        "comb_Wt_x": np.ascontiguousarray(np.asarray(comb_W, f32).T[:O], f32),
        "comb_Wt_c": np.ascontiguousarray(np.asarray(comb_W, f32).T[O:], f32),
        "comb_b": np.ascontiguousarray(np.asarray(comb_b, f32)[:, None], f32),
        "W_ih_t": np.ascontiguousarray(np.asarray(W_ih, f32).T, f32),
        "W_hh_t": np.ascontiguousarray(np.asarray(W_hh, f32).T, f32),
        "gbias": np.ascontiguousarray(gbias.T, f32),  # [H, 4]
        "out_Wt": np.ascontiguousarray(np.asarray(out_W, f32).T, f32),
        "out_b": np.ascontiguousarray(np.asarray(out_b, f32)[:, None], f32),
    }
    in_maps = []
    for cidx in range(n_cores):
        rows = slice(cidx * bc, (cidx + 1) * bc)
        m = dict(weights)
        m["xT"] = np.ascontiguousarray(x[rows].T, f32)
        m["hT"] = np.ascontiguousarray(h0[rows].T, f32)
        m["cT"] = np.ascontiguousarray(c0[rows].T, f32)
        m["enc"] = enc[rows]
        in_maps.append(m)
    return in_maps


def _gather(results, bc: int = BC):
    out = np.concatenate([r["outT"].T for r in results], axis=0)
    h1 = np.concatenate([r["h1T"].T for r in results], axis=0)
    c1 = np.concatenate([r["c1T"].T for r in results], axis=0)
    attn = np.concatenate([r["attn"] for r in results], axis=0)
    return (
        np.ascontiguousarray(out, np.float32),
        np.ascontiguousarray(h1, np.float32)[None],
        np.ascontiguousarray(c1, np.float32)[None],
        np.ascontiguousarray(attn, np.float32),
    )


def _install_ntff_hook():
    """The image's antenv lacks axon_hooks; provide it and register the
    ctypes NTFF profiling hook so trace=True yields exec_time_ns."""
    import sys
    import types

    if "antenv.axon_hooks" in sys.modules:
        return
    import antenv

    mod = types.ModuleType("antenv.axon_hooks")
    _holder = {"hook": None}
    mod.set_axon_ntff_profile_hook = lambda h: _holder.__setitem__("hook", h)
    mod.get_axon_ntff_profile_hook = lambda: _holder["hook"]
    sys.modules["antenv.axon_hooks"] = mod
    antenv.axon_hooks = mod
    try:
        from trn_agent_boot.trn_boot import _ntff_profile_via_ctypes

        hook = _ntff_profile_via_ctypes("/opt/axon/libaxon_pjrt.so")
        if hook is not None:
            mod.set_axon_ntff_profile_hook(hook)
    except Exception as e:  # degrade: tracing skipped, run still works
        print(f"ntff hook install failed: {e}")


def run(inputs: dict, trace: bool = False, bc: int = BC, n_cores: int = N_CORES):
    if trace:
        _install_ntff_hook()
    key = (bc, n_cores)
    if key not in _NC_CACHE:
        _NC_CACHE[key] = build_nc(bc=bc)
    nc = _NC_CACHE[key]
    in_maps = _prep_in_maps(**inputs, bc=bc, n_cores=n_cores)
    res = run_bass_kernel_spmd(
        nc, in_maps, list(range(n_cores)), trace=trace,
        tmpdir=os.environ.get("BASS_TRACE_DIR"),
    )
    return _gather(res.results, bc=bc), res


def kernel(**inputs):
    outputs, _ = run(inputs)
    return outputs


# revision 9
# speedup vs baseline: 1.0034x; 1.0009x over previous
"""Trainium2 Bass kernel for nn_AttnDecoderLSTM (B=4096, S=512, H=O=128).

Data-parallel over 8 NeuronCores: each core owns 512 batch rows.
All on-chip compute runs in transposed (feature-on-partition) layout;
the host pre-transposes the small per-core activations and weights and
un-transposes the small outputs.
"""

import os
import sys

sys.path.insert(0, "/opt/trn_rl_repo")

import numpy as np

import concourse.bass as bass  # noqa: F401  (AP types)
import concourse.tile as tile
from concourse import bacc, mybir
from concourse.bass_utils import run_bass_kernel_spmd
from concourse.masks import make_identity

F32 = mybir.dt.float32
AF = mybir.ActivationFunctionType

B, S, H, O = 4096, 512, 128, 128
N_CORES = 8
BC = B // N_CORES  # batch rows per core (512)
P = 128  # tile size along batch
NCH = S // 128  # s-chunks (4)
NB = int(os.environ.get("K_NB", "4"))  # batch rows per encoder DMA
MATVEC_DT = os.environ.get("K_MVDT", "bf16")  # "f32" | "f32r" | "bf16"
NW = int(os.environ.get("K_NW", "0"))  # HAM-warmer cadence in batch rows (0 = off)
EPOOL = int(os.environ.get("K_EPOOL", "12"))  # encoder tile double-buffer depth
BF16 = mybir.dt.bfloat16

_NC_CACHE: dict = {}


def build_nc(bc: int = BC, nb: int = NB, mv_dt: str = MATVEC_DT, nw: int = NW):
    """Build the per-core Bass program (same program on all cores)."""
    nc = bacc.Bacc("TRN2", target_bir_lowering=False)

    # ---- DRAM I/O (per-core shard shapes) ----
    xT_d = nc.dram_tensor("xT", [O, bc], F32, kind="ExternalInput")
    hT_d = nc.dram_tensor("hT", [H, bc], F32, kind="ExternalInput")
    cT_d = nc.dram_tensor("cT", [H, bc], F32, kind="ExternalInput")
    enc_d = nc.dram_tensor("enc", [bc, S, H], F32, kind="ExternalInput")

    attn_Wt_x_d = nc.dram_tensor("attn_Wt_x", [O, S], F32, kind="ExternalInput")
    attn_Wt_h_d = nc.dram_tensor("attn_Wt_h", [H, S], F32, kind="ExternalInput")
    attn_b_d = nc.dram_tensor("attn_b", [1, S], F32, kind="ExternalInput")
    comb_Wt_x_d = nc.dram_tensor("comb_Wt_x", [O, O], F32, kind="ExternalInput")
    comb_Wt_c_d = nc.dram_tensor("comb_Wt_c", [H, O], F32, kind="ExternalInput")
    comb_b_d = nc.dram_tensor("comb_b", [O, 1], F32, kind="ExternalInput")
    W_ih_t_d = nc.dram_tensor("W_ih_t", [O, 4 * H], F32, kind="ExternalInput")
    W_hh_t_d = nc.dram_tensor("W_hh_t", [H, 4 * H], F32, kind="ExternalInput")
    gbias_d = nc.dram_tensor("gbias", [H, 4], F32, kind="ExternalInput")
    out_Wt_d = nc.dram_tensor("out_Wt", [H, O], F32, kind="ExternalInput")
    out_b_d = nc.dram_tensor("out_b", [O, 1], F32, kind="ExternalInput")

    outT_d = nc.dram_tensor("outT", [O, bc], F32, kind="ExternalOutput")
    h1T_d = nc.dram_tensor("h1T", [H, bc], F32, kind="ExternalOutput")
    c1T_d = nc.dram_tensor("c1T", [H, bc], F32, kind="ExternalOutput")
    attn_d = nc.dram_tensor("attn", [bc, S], F32, kind="ExternalOutput")
    hamd_d = (
        nc.dram_tensor("hamd", [128, 4], F32, kind="ExternalOutput") if nw else None
    )

    ntiles = bc // P

    with tile.TileContext(nc) as tc:
        with (
            tc.tile_pool(name="wpool", bufs=1) as wp,
            tc.tile_pool(name="epool", bufs=EPOOL) as ep,
            tc.tile_pool(name="xpool", bufs=2) as xp,
            tc.tile_pool(name="apool", bufs=2) as ap,
            tc.tile_pool(name="spool", bufs=2) as sp,
            tc.tile_pool(name="ps_logit", bufs=2, space="PSUM") as ps_logit,
            tc.tile_pool(name="ps_ctx", bufs=2, space="PSUM") as ps_ctx,
            tc.tile_pool(name="ps_small", bufs=3, space="PSUM") as ps_small,
            tc.tile_pool(name="ps_ham", bufs=1, space="PSUM") as ps_ham,
        ):
            # ---- constants / weights (loaded once) ----
            ident = wp.tile([128, 128], F32)
            make_identity(nc, ident[:])
            ones_row = wp.tile([1, P], F32)
            nc.gpsimd.memset(ones_row[:], 1.0)
            if nw:
                zz = wp.tile([128, 512], BF16)
                nc.gpsimd.memset(zz[:], 0.0)
                ham_ps = ps_ham.tile([128, 512], F32)

            def wload(dram, shape):
                t = wp.tile(shape, F32, tag=dram.name)
                nc.scalar.dma_start(t[:], dram[:])
                return t

            attn_Wt_x = wload(attn_Wt_x_d, [O, S])
            attn_Wt_h = wload(attn_Wt_h_d, [H, S])
            attn_b = wload(attn_b_d, [1, S])
            comb_Wt_x = wload(comb_Wt_x_d, [O, O])
            comb_Wt_c = wload(comb_Wt_c_d, [H, O])
            comb_b = wload(comb_b_d, [O, 1])
            W_ih_t = wload(W_ih_t_d, [O, 4 * H])
            W_hh_t = wload(W_hh_t_d, [H, 4 * H])
            gbias = wload(gbias_d, [H, 4])
            out_Wt = wload(out_Wt_d, [H, O])
            out_b = wload(out_b_d, [O, 1])

            at_dt = BF16 if mv_dt == "bf16" else F32
            e_dt = BF16 if mv_dt == "bf16" else F32

            xT_all = wp.tile([O, bc], F32)
            hT_all = wp.tile([H, bc], F32)
            cT_all = wp.tile([H, bc], F32)
            nc.scalar.dma_start(xT_all[:], xT_d[:])
            nc.scalar.dma_start(hT_all[:], hT_d[:])
            nc.scalar.dma_start(cT_all[:], cT_d[:])
            h1_all = wp.tile([H, bc], F32)
            c1_all = wp.tile([H, bc], F32)
            out_all = wp.tile([O, bc], F32)

            def stage_a(t):
                """Inputs, attention logits, softmax, transposed attn columns."""
                tsl = slice(t * P, (t + 1) * P)
                xT = xT_all[:, tsl]
                hT = hT_all[:, tsl]
                cT = cT_all[:, tsl]

                logits = ps_logit.tile([P, S], F32)
                nc.tensor.matmul(logits[:], xT, attn_Wt_x[:], start=True, stop=False)
                nc.tensor.matmul(logits[:], hT, attn_Wt_h[:], start=False, stop=False)
                nc.tensor.matmul(
                    logits[:], ones_row[:], attn_b[:], start=False, stop=True
                )

                negmax = ap.tile([P, 1], F32, tag="negmax")
                nc.vector.reduce_max(
                    negmax[:], logits[:], axis=mybir.AxisListType.X, negate=True
                )
                A = ap.tile([P, S], F32, tag="A")
                esum = ap.tile([P, 1], F32, tag="esum")
                nc.scalar.activation(
                    A[:], logits[:], AF.Exp, bias=negmax[:], accum_out=esum[:]
                )
                rs = ap.tile([P, 1], F32, tag="rs")
                nc.vector.reciprocal(rs[:], esum[:])
                nc.vector.tensor_scalar_mul(A[:], A[:], rs[:])
                nc.scalar.dma_start(attn_d[tsl, :], A[:])

                ATs = []
                for c in range(NCH):
                    ps = ps_small.tile([128, P], F32, tag="ps_s")
                    nc.tensor.transpose(ps[:], A[:, c::NCH], ident[:])
                    ATc = ap.tile([128, P], at_dt, tag=f"AT{c}")
                    nc.scalar.copy(ATc[:], ps[:])
                    ATs.append(ATc)
                return dict(xT=xT, hT=hT, cT=cT, ATs=ATs)

            def stage_bcde(t, st):
                tsl = slice(t * P, (t + 1) * P)
                xT, hT, cT, ATs = st["xT"], st["hT"], st["cT"], st["ATs"]

                # ---- stage B: ctx^T accumulation (the heavy stream) ----
                ctxT_ps = ps_ctx.tile([H, P], F32)
                for bb in range(0, P, nb):
                    if nw and bb % nw == 0:
                        nc.tensor.matmul(
                            ham_ps[:], zz[:, :128], zz[:], start=True, stop=True
                        )
                    et = ep.tile([128, nb, NCH, H], e_dt, tag="E")
                    esrc = enc_d[t * P + bb : t * P + bb + nb, :, :].rearrange(
                        "bb (p j) h -> p bb j h", j=NCH
                    )
                    if mv_dt == "bf16":
                        nc.gpsimd.dma_start(et[:], esrc)
                    else:
                        nc.sync.dma_start(et[:], esrc)
                    for j in range(nb):
                        col = slice(bb + j, bb + j + 1)
                        for c in range(NCH):
                            nc.tensor.matmul(
                                ctxT_ps[:, col],
                                et[:, j, c, :],
                                ATs[c][:, col],
                                start=(c == 0),
                                stop=(c == NCH - 1),
                            )
                ctxT = sp.tile([H, P], F32, tag="ctxT")
                nc.scalar.copy(ctxT[:], ctxT_ps[:])

                # ---- stage C: comb = relu([x, ctx] @ comb_W.T + comb_b) ----
                comb_ps = ps_small.tile([O, P], F32, tag="ps_s")
                nc.tensor.matmul(comb_ps[:], comb_Wt_x[:], xT, start=True, stop=False)
                nc.tensor.matmul(
                    comb_ps[:], comb_Wt_c[:], ctxT[:], start=False, stop=True
                )
                combT = sp.tile([O, P], F32, tag="combT")
                nc.scalar.activation(combT[:], comb_ps[:], AF.Relu, bias=comb_b[:])

                # ---- stage D: LSTM cell (gates: 0=i, 1=f, 2=g, 3=o) ----
                gact = []
                for g in range(4):
                    gsl = slice(g * H, (g + 1) * H)
                    gps = ps_small.tile([H, P], F32, tag="ps_s")
                    nc.tensor.matmul(
                        gps[:], W_ih_t[:, gsl], combT[:], start=True, stop=False
                    )
                    nc.tensor.matmul(
                        gps[:], W_hh_t[:, gsl], hT, start=False, stop=True
                    )
                    gs = sp.tile([H, P], F32, tag=f"g{g}")
                    if g == 2:
                        nc.scalar.activation(gs[:], gps[:], AF.Tanh, bias=gbias[:, 2:3])
                    else:
                        # sigmoid(x+b) = 0.5*tanh(0.5x+0.5b)+0.5
                        nc.scalar.activation(
                            gs[:], gps[:], AF.Tanh, bias=gbias[:, g : g + 1], scale=0.5
                        )
                        nc.vector.tensor_scalar(
                            gs[:], gs[:], 0.5, 0.5,
                            op0=mybir.AluOpType.mult, op1=mybir.AluOpType.add,
                        )
                    gact.append(gs)

                i_g, f_g, g_g, o_g = gact
                t1 = sp.tile([H, P], F32, tag="t1")
                nc.vector.tensor_mul(t1[:], f_g[:], cT)
                t2 = sp.tile([H, P], F32, tag="t2")
                nc.vector.tensor_mul(t2[:], i_g[:], g_g[:])
                c1T = c1_all[:, tsl]
                nc.vector.tensor_add(c1T, t1[:], t2[:])

                tc1 = sp.tile([H, P], F32, tag="tc1")
                nc.scalar.activation(tc1[:], c1T, AF.Tanh)
                h1T = h1_all[:, tsl]
                nc.vector.tensor_mul(h1T, o_g[:], tc1[:])

                # ---- stage E: out = h1 @ out_W.T + out_b ----
                out_ps = ps_small.tile([O, P], F32, tag="ps_s")
                nc.tensor.matmul(out_ps[:], out_Wt[:], h1T, start=True, stop=True)
                nc.scalar.activation(
                    out_all[:, tsl], out_ps[:], AF.Identity, bias=out_b[:]
                )

            # Software pipeline: stage A of tile t+1 is emitted before the
            # heavy stream of tile t so the PE has the next tile's attention
            # columns ready before its matvecs begin.
            st = stage_a(0)
            for t in range(ntiles):
                nxt = stage_a(t + 1) if t + 1 < ntiles else None
                stage_bcde(t, st)
                st = nxt

            nc.scalar.dma_start(h1T_d[:], h1_all[:])
            nc.scalar.dma_start(c1T_d[:], c1_all[:])
            nc.scalar.dma_start(outT_d[:], out_all[:])

            if nw:
                hs = wp.tile([128, 4], F32)
                nc.vector.tensor_copy(hs[:], ham_ps[:, :4])
                nc.scalar.dma_start(hamd_d[:], hs[:])

    nc.compile()
    return nc


def _prep_in_maps(
    input, h, c, encoder_outputs, attn_W, attn_b, comb_W, comb_b,
    W_ih, b_ih, W_hh, b_hh, out_W, out_b, bc: int = BC, n_cores: int = N_CORES,
):
    f32 = np.float32
    x = np.ascontiguousarray(input[:, 0, :], dtype=f32)  # [B, O]
    h0 = np.ascontiguousarray(h[0], dtype=f32)  # [B, H]
    c0 = np.ascontiguousarray(c[0], dtype=f32)  # [B, H]
    enc = np.ascontiguousarray(encoder_outputs, dtype=f32)

    gb = (np.asarray(b_ih, f32) + np.asarray(b_hh, f32)).reshape(4, H)
    gbias = gb.copy()
    for g in (0, 1, 3):
        gbias[g] *= 0.5  # folded into the 0.5*tanh(0.5x+0.5b)+0.5 sigmoid
    weights = {
        "attn_Wt_x": np.ascontiguousarray(np.asarray(attn_W, f32).T[:O], f32),
        "attn_Wt_h": np.ascontiguousarray(np.asarray(attn_W, f32).T[O:], f32),
        "attn_b": np.ascontiguousarray(np.asarray(attn_b, f32)[None, :], f32),
        "comb_Wt_x": np.ascontiguousarray(np.asarray(comb_W, f32).T[:O], f32),
        "comb_Wt_c": np.ascontiguousarray(np.asarray(comb_W, f32).T[O:], f32),
        "comb_b": np.ascontiguousarray(np.asarray(comb_b, f32)[:, None], f32),
        "W_ih_t": np.ascontiguousarray(np.asarray(W_ih, f32).T, f32),
        "W_hh_t": np.ascontiguousarray(np.asarray(W_hh, f32).T, f32),
        "gbias": np.ascontiguousarray(gbias.T, f32),  # [H, 4]
        "out_Wt": np.ascontiguousarray(np.asarray(out_W, f32).T, f32),
        "out_b": np.ascontiguousarray(np.asarray(out_b, f32)[:, None], f32),
    }
    in_maps = []
    for cidx in range(n_cores):
        rows = slice(cidx * bc, (cidx + 1) * bc)
        m = dict(weights)
        m["xT"] = np.ascontiguousarray(x[rows].T, f32)
        m["hT"] = np.ascontiguousarray(h0[rows].T, f32)
        m["cT"] = np.ascontiguousarray(c0[rows].T, f32)
        m["enc"] = enc[rows]
        in_maps.append(m)
    return in_maps


def _gather(results, bc: int = BC):
    out = np.concatenate([r["outT"].T for r in results], axis=0)
    h1 = np.concatenate([r["h1T"].T for r in results], axis=0)
    c1 = np.concatenate([r["c1T"].T for r in results], axis=0)
    attn = np.concatenate([r["attn"] for r in results], axis=0)
    return (
        np.ascontiguousarray(out, np.float32),
        np.ascontiguousarray(h1, np.float32)[None],
        np.ascontiguousarray(c1, np.float32)[None],
        np.ascontiguousarray(attn, np.float32),
    )


def _install_ntff_hook():
    """The image's antenv lacks axon_hooks; provide it and register the
    ctypes NTFF profiling hook so trace=True yields exec_time_ns."""
    import sys
    import types

    if "antenv.axon_hooks" in sys.modules:
        return
    import antenv

    mod = types.ModuleType("antenv.axon_hooks")
    _holder = {"hook": None}
    mod.set_axon_ntff_profile_hook = lambda h: _holder.__setitem__("hook", h)
    mod.get_axon_ntff_profile_hook = lambda: _holder["hook"]
    sys.modules["antenv.axon_hooks"] = mod
    antenv.axon_hooks = mod
    try:
        from trn_agent_boot.trn_boot import _ntff_profile_via_ctypes

        hook = _ntff_profile_via_ctypes("/opt/axon/libaxon_pjrt.so")
        if hook is not None:
            mod.set_axon_ntff_profile_hook(hook)
    except Exception as e:  # degrade: tracing skipped, run still works
        print(f"ntff hook install failed: {e}")


def run(inputs: dict, trace: bool = False, bc: int = BC, n_cores: int = N_CORES):
    if trace:
        _install_ntff_hook()
    key = (bc, n_cores)
    if key not in _NC_CACHE:
        _NC_CACHE[key] = build_nc(bc=bc)
    nc = _NC_CACHE[key]
    in_maps = _prep_in_maps(**inputs, bc=bc, n_cores=n_cores)
    last_err = None
    for attempt in range(2):
        try:
            res = run_bass_kernel_spmd(
                nc, in_maps, list(range(n_cores)), trace=trace,
                tmpdir=os.environ.get("BASS_TRACE_DIR"),
            )
            return _gather(res.results, bc=bc), res
        except Exception as e:  # transient device/tunnel flakes: retry once
            last_err = e
    raise last_err


def kernel(**inputs):
    outputs, _ = run(inputs)
    return outputs


# revision 10
# speedup vs baseline: 1.0907x; 1.0870x over previous
"""Trainium2 Bass kernel for nn_AttnDecoderLSTM (B=4096, S=512, H=O=128).

Data-parallel over 8 NeuronCores: each core owns 512 batch rows.
All on-chip compute runs in transposed (feature-on-partition) layout;
the host pre-transposes the small per-core activations and weights and
un-transposes the small outputs.
"""

import os
import sys

sys.path.insert(0, "/opt/trn_rl_repo")

import numpy as np

import concourse.bass as bass  # noqa: F401  (AP types)
import concourse.tile as tile
from concourse import bacc, mybir
from concourse.bass_utils import run_bass_kernel_spmd

F32 = mybir.dt.float32
AF = mybir.ActivationFunctionType

B, S, H, O = 4096, 512, 128, 128
N_CORES = 8
BC = B // N_CORES  # batch rows per core (512)
P = 128  # tile size along batch
NCH = S // 128  # s-chunks (4)
NB = int(os.environ.get("K_NB", "4"))  # batch rows per encoder DMA
MATVEC_DT = os.environ.get("K_MVDT", "bf16")  # "f32" | "f32r" | "bf16"
NW = int(os.environ.get("K_NW", "0"))  # HAM-warmer cadence in batch rows (0 = off)
EPOOL = int(os.environ.get("K_EPOOL", "12"))  # encoder tile double-buffer depth
BF16 = mybir.dt.bfloat16

_NC_CACHE: dict = {}


def build_nc(bc: int = BC, nb: int = NB, mv_dt: str = MATVEC_DT, nw: int = NW):
    """Build the per-core Bass program (same program on all cores)."""
    nc = bacc.Bacc("TRN2", target_bir_lowering=False)

    # ---- DRAM I/O (per-core shard shapes) ----
    xT_d = nc.dram_tensor("xT", [O, bc], F32, kind="ExternalInput")
    hT_d = nc.dram_tensor("hT", [H, bc], F32, kind="ExternalInput")
    cT_d = nc.dram_tensor("cT", [H, bc], F32, kind="ExternalInput")
    enc_d = nc.dram_tensor("enc", [bc, S, H], F32, kind="ExternalInput")

    attn_Wt_x_d = nc.dram_tensor("attn_Wt_x", [O, S], F32, kind="ExternalInput")
    attn_Wt_h_d = nc.dram_tensor("attn_Wt_h", [H, S], F32, kind="ExternalInput")
    aux_row_d = nc.dram_tensor("aux_row", [1, S + P], F32, kind="ExternalInput")
    comb_Wt_x_d = nc.dram_tensor("comb_Wt_x", [O, O], F32, kind="ExternalInput")
    comb_Wt_c_d = nc.dram_tensor("comb_Wt_c", [H, O], F32, kind="ExternalInput")
    wpack_d = nc.dram_tensor("wpack", [128, 134], F32, kind="ExternalInput")
    W_ih_t_d = nc.dram_tensor("W_ih_t", [O, 4 * H], F32, kind="ExternalInput")
    W_hh_t_d = nc.dram_tensor("W_hh_t", [H, 4 * H], F32, kind="ExternalInput")
    out_Wt_d = nc.dram_tensor("out_Wt", [H, O], F32, kind="ExternalInput")

    outT_d = nc.dram_tensor("outT", [O, bc], F32, kind="ExternalOutput")
    h1T_d = nc.dram_tensor("h1T", [H, bc], F32, kind="ExternalOutput")
    c1T_d = nc.dram_tensor("c1T", [H, bc], F32, kind="ExternalOutput")
    attn_d = nc.dram_tensor("attn", [bc, S], F32, kind="ExternalOutput")
    hamd_d = (
        nc.dram_tensor("hamd", [128, 4], F32, kind="ExternalOutput") if nw else None
    )

    ntiles = bc // P

    with tile.TileContext(nc) as tc:
        with (
            tc.tile_pool(name="wpool", bufs=1) as wp,
            tc.tile_pool(name="epool", bufs=EPOOL) as ep,
            tc.tile_pool(name="xpool", bufs=2) as xp,
            tc.tile_pool(name="apool", bufs=2) as ap,
            tc.tile_pool(name="spool", bufs=2) as sp,
            tc.tile_pool(name="ps_logit", bufs=2, space="PSUM") as ps_logit,
            tc.tile_pool(name="ps_ctx", bufs=2, space="PSUM") as ps_ctx,
            tc.tile_pool(name="ps_small", bufs=3, space="PSUM") as ps_small,
            tc.tile_pool(name="ps_ham", bufs=1, space="PSUM") as ps_ham,
        ):
            # ---- constants / weights (loaded once) ----
            # wpack columns: 0:128 identity, 128 comb_b, 129 out_b, 130:134 gbias
            wpack = wp.tile([128, 134], F32)
            nc.scalar.dma_start(wpack[:], wpack_d[:])
            ident = wpack[:, 0:128]
            comb_b = wpack[:, 128:129]
            out_b = wpack[:, 129:130]
            gbias = wpack[:, 130:134]
            aux_row = wp.tile([1, S + P], F32)
            nc.scalar.dma_start(aux_row[:], aux_row_d[:])
            attn_b = aux_row[:, :S]
            ones_row = aux_row[:, S : S + P]
            if nw:
                zz = wp.tile([128, 512], BF16)
                nc.gpsimd.memset(zz[:], 0.0)
                ham_ps = ps_ham.tile([128, 512], F32)

            def wload(dram, shape):
                t = wp.tile(shape, F32, tag=dram.name)
                nc.scalar.dma_start(t[:], dram[:])
                return t

            attn_Wt_x = wload(attn_Wt_x_d, [O, S])
            attn_Wt_h = wload(attn_Wt_h_d, [H, S])
            comb_Wt_x = wload(comb_Wt_x_d, [O, O])
            comb_Wt_c = wload(comb_Wt_c_d, [H, O])
            W_ih_t = wload(W_ih_t_d, [O, 4 * H])
            W_hh_t = wload(W_hh_t_d, [H, 4 * H])
            out_Wt = wload(out_Wt_d, [H, O])

            at_dt = BF16 if mv_dt == "bf16" else F32
            e_dt = BF16 if mv_dt == "bf16" else F32

            xT_all = wp.tile([O, bc], F32)
            hT_all = wp.tile([H, bc], F32)
            cT_all = wp.tile([H, bc], F32)
            nc.scalar.dma_start(xT_all[:], xT_d[:])
            nc.scalar.dma_start(hT_all[:], hT_d[:])
            nc.scalar.dma_start(cT_all[:], cT_d[:])
            h1_all = wp.tile([H, bc], F32)
            c1_all = wp.tile([H, bc], F32)
            out_all = wp.tile([O, bc], F32)

            def stage_a(t):
                """Inputs, attention logits, softmax, transposed attn columns."""
                tsl = slice(t * P, (t + 1) * P)
                xT = xT_all[:, tsl]
                hT = hT_all[:, tsl]
                cT = cT_all[:, tsl]

                logits = ps_logit.tile([P, S], F32)
                nc.tensor.matmul(logits[:], xT, attn_Wt_x[:], start=True, stop=False)
                nc.tensor.matmul(logits[:], hT, attn_Wt_h[:], start=False, stop=False)
                nc.tensor.matmul(
                    logits[:], ones_row, attn_b, start=False, stop=True
                )

                negmax = ap.tile([P, 1], F32, tag="negmax")
                nc.vector.reduce_max(
                    negmax[:], logits[:], axis=mybir.AxisListType.X, negate=True
                )
                A = ap.tile([P, S], F32, tag="A")
                esum = ap.tile([P, 1], F32, tag="esum")
                nc.scalar.activation(
                    A[:], logits[:], AF.Exp, bias=negmax[:], accum_out=esum[:]
                )
                rs = ap.tile([P, 1], F32, tag="rs")
                nc.vector.reciprocal(rs[:], esum[:])
                nc.vector.tensor_scalar_mul(A[:], A[:], rs[:])
                nc.scalar.dma_start(attn_d[tsl, :], A[:])

                ATs = []
                for c in range(NCH):
                    ps = ps_small.tile([128, P], F32, tag="ps_s")
                    nc.tensor.transpose(ps[:], A[:, c::NCH], ident)
                    ATc = ap.tile([128, P], at_dt, tag=f"AT{c}")
                    nc.scalar.copy(ATc[:], ps[:])
                    ATs.append(ATc)
                return dict(xT=xT, hT=hT, cT=cT, ATs=ATs)

            def stage_bcde(t, st):
                tsl = slice(t * P, (t + 1) * P)
                xT, hT, cT, ATs = st["xT"], st["hT"], st["cT"], st["ATs"]

                # ---- stage B: ctx^T accumulation (the heavy stream) ----
                ctxT_ps = ps_ctx.tile([H, P], F32)
                for bb in range(0, P, nb):
                    if nw and bb % nw == 0:
                        nc.tensor.matmul(
                            ham_ps[:], zz[:, :128], zz[:], start=True, stop=True
                        )
                    et = ep.tile([128, nb, NCH, H], e_dt, tag="E")
                    esrc = enc_d[t * P + bb : t * P + bb + nb, :, :].rearrange(
                        "bb (p j) h -> p bb j h", j=NCH
                    )
                    if mv_dt == "bf16":
                        nc.gpsimd.dma_start(et[:], esrc)
                    else:
                        nc.sync.dma_start(et[:], esrc)
                    for j in range(nb):
                        col = slice(bb + j, bb + j + 1)
                        for c in range(NCH):
                            nc.tensor.matmul(
                                ctxT_ps[:, col],
                                et[:, j, c, :],
                                ATs[c][:, col],
                                start=(c == 0),
                                stop=(c == NCH - 1),
                            )
                ctxT = sp.tile([H, P], F32, tag="ctxT")
                nc.scalar.copy(ctxT[:], ctxT_ps[:])

                # ---- stage C: comb = relu([x, ctx] @ comb_W.T + comb_b) ----
                comb_ps = ps_small.tile([O, P], F32, tag="ps_s")
                nc.tensor.matmul(comb_ps[:], comb_Wt_x[:], xT, start=True, stop=False)
                nc.tensor.matmul(
                    comb_ps[:], comb_Wt_c[:], ctxT[:], start=False, stop=True
                )
                combT = sp.tile([O, P], F32, tag="combT")
                nc.scalar.activation(combT[:], comb_ps[:], AF.Relu, bias=comb_b)

                # ---- stage D: LSTM cell (gates: 0=i, 1=f, 2=g, 3=o) ----
                gact = []
                for g in range(4):
                    gsl = slice(g * H, (g + 1) * H)
                    gps = ps_small.tile([H, P], F32, tag="ps_s")
                    nc.tensor.matmul(
                        gps[:], W_ih_t[:, gsl], combT[:], start=True, stop=False
                    )
                    nc.tensor.matmul(
                        gps[:], W_hh_t[:, gsl], hT, start=False, stop=True
                    )
                    gs = sp.tile([H, P], F32, tag=f"g{g}")
                    if g == 2:
                        nc.scalar.activation(gs[:], gps[:], AF.Tanh, bias=gbias[:, 2:3])
                    else:
                        # sigmoid(x+b) = 0.5*tanh(0.5x+0.5b)+0.5
                        nc.scalar.activation(
                            gs[:], gps[:], AF.Tanh, bias=gbias[:, g : g + 1], scale=0.5
                        )
                        nc.vector.tensor_scalar(
                            gs[:], gs[:], 0.5, 0.5,
                            op0=mybir.AluOpType.mult, op1=mybir.AluOpType.add,
                        )
                    gact.append(gs)

                i_g, f_g, g_g, o_g = gact
                t1 = sp.tile([H, P], F32, tag="t1")
                nc.vector.tensor_mul(t1[:], f_g[:], cT)
                t2 = sp.tile([H, P], F32, tag="t2")
                nc.vector.tensor_mul(t2[:], i_g[:], g_g[:])
                c1T = c1_all[:, tsl]
                nc.vector.tensor_add(c1T, t1[:], t2[:])

                tc1 = sp.tile([H, P], F32, tag="tc1")
                nc.scalar.activation(tc1[:], c1T, AF.Tanh)
                h1T = h1_all[:, tsl]
                nc.vector.tensor_mul(h1T, o_g[:], tc1[:])

                # ---- stage E: out = h1 @ out_W.T + out_b ----
                out_ps = ps_small.tile([O, P], F32, tag="ps_s")
                nc.tensor.matmul(out_ps[:], out_Wt[:], h1T, start=True, stop=True)
                nc.scalar.activation(
                    out_all[:, tsl], out_ps[:], AF.Identity, bias=out_b
                )

            # Software pipeline: stage A of tile t+1 is emitted before the
            # heavy stream of tile t so the PE has the next tile's attention
            # columns ready before its matvecs begin.
            st = stage_a(0)
            for t in range(ntiles):
                nxt = stage_a(t + 1) if t + 1 < ntiles else None
                stage_bcde(t, st)
                st = nxt

            nc.scalar.dma_start(h1T_d[:], h1_all[:])
            nc.scalar.dma_start(c1T_d[:], c1_all[:])
            nc.scalar.dma_start(outT_d[:], out_all[:])

            if nw:
                hs = wp.tile([128, 4], F32)
                nc.vector.tensor_copy(hs[:], ham_ps[:, :4])
                nc.scalar.dma_start(hamd_d[:], hs[:])

    nc.compile()
    return nc


def _prep_in_maps(
    input, h, c, encoder_outputs, attn_W, attn_b, comb_W, comb_b,
    W_ih, b_ih, W_hh, b_hh, out_W, out_b, bc: int = BC, n_cores: int = N_CORES,
):
    f32 = np.float32
    x = np.ascontiguousarray(input[:, 0, :], dtype=f32)  # [B, O]
    h0 = np.ascontiguousarray(h[0], dtype=f32)  # [B, H]
    c0 = np.ascontiguousarray(c[0], dtype=f32)  # [B, H]
    enc = np.ascontiguousarray(encoder_outputs, dtype=f32)

    gb = (np.asarray(b_ih, f32) + np.asarray(b_hh, f32)).reshape(4, H)
    gbias = gb.copy()
    for g in (0, 1, 3):
        gbias[g] *= 0.5  # folded into the 0.5*tanh(0.5x+0.5b)+0.5 sigmoid
    wpack = np.zeros((128, 134), f32)
    wpack[:, 0:128] = np.eye(128, dtype=f32)
    wpack[:, 128] = np.asarray(comb_b, f32)
    wpack[:, 129] = np.asarray(out_b, f32)
    wpack[:, 130:134] = gbias.T
    aux_row = np.zeros((1, S + 128), f32)
    aux_row[0, :S] = np.asarray(attn_b, f32)
    aux_row[0, S:] = 1.0
    weights = {
        "attn_Wt_x": np.ascontiguousarray(np.asarray(attn_W, f32).T[:O], f32),
        "attn_Wt_h": np.ascontiguousarray(np.asarray(attn_W, f32).T[O:], f32),
        "aux_row": aux_row,
        "comb_Wt_x": np.ascontiguousarray(np.asarray(comb_W, f32).T[:O], f32),
        "comb_Wt_c": np.ascontiguousarray(np.asarray(comb_W, f32).T[O:], f32),
        "W_ih_t": np.ascontiguousarray(np.asarray(W_ih, f32).T, f32),
        "W_hh_t": np.ascontiguousarray(np.asarray(W_hh, f32).T, f32),
        "wpack": wpack,
        "out_Wt": np.ascontiguousarray(np.asarray(out_W, f32).T, f32),
    }
    in_maps = []
    for cidx in range(n_cores):
        rows = slice(cidx * bc, (cidx + 1) * bc)
        m = dict(weights)
        m["xT"] = np.ascontiguousarray(x[rows].T, f32)
        m["hT"] = np.ascontiguousarray(h0[rows].T, f32)
        m["cT"] = np.ascontiguousarray(c0[rows].T, f32)
        m["enc"] = enc[rows]
        in_maps.append(m)
    return in_maps


def _gather(results, bc: int = BC):
    out = np.concatenate([r["outT"].T for r in results], axis=0)
    h1 = np.concatenate([r["h1T"].T for r in results], axis=0)
    c1 = np.concatenate([r["c1T"].T for r in results], axis=0)
    attn = np.concatenate([r["attn"] for r in results], axis=0)
    return (
        np.ascontiguousarray(out, np.float32),
        np.ascontiguousarray(h1, np.float32)[None],
        np.ascontiguousarray(c1, np.float32)[None],
        np.ascontiguousarray(attn, np.float32),
    )


def _install_ntff_hook():
    """The image's antenv lacks axon_hooks; provide it and register the
    ctypes NTFF profiling hook so trace=True yields exec_time_ns."""
    import sys
    import types

    if "antenv.axon_hooks" in sys.modules:
        return
    import antenv

    mod = types.ModuleType("antenv.axon_hooks")
    _holder = {"hook": None}
    mod.set_axon_ntff_profile_hook = lambda h: _holder.__setitem__("hook", h)
    mod.get_axon_ntff_profile_hook = lambda: _holder["hook"]
    sys.modules["antenv.axon_hooks"] = mod
    antenv.axon_hooks = mod
    try:
        from trn_agent_boot.trn_boot import _ntff_profile_via_ctypes

        hook = _ntff_profile_via_ctypes("/opt/axon/libaxon_pjrt.so")
        if hook is not None:
            mod.set_axon_ntff_profile_hook(hook)
    except Exception as e:  # degrade: tracing skipped, run still works
        print(f"ntff hook install failed: {e}")


def run(inputs: dict, trace: bool = False, bc: int = BC, n_cores: int = N_CORES):
    if trace:
        _install_ntff_hook()
    key = (bc, n_cores)
    if key not in _NC_CACHE:
        _NC_CACHE[key] = build_nc(bc=bc)
    nc = _NC_CACHE[key]
    in_maps = _prep_in_maps(**inputs, bc=bc, n_cores=n_cores)
    last_err = None
    for attempt in range(2):
        try:
            res = run_bass_kernel_spmd(
                nc, in_maps, list(range(n_cores)), trace=trace,
                tmpdir=os.environ.get("BASS_TRACE_DIR"),
            )
            return _gather(res.results, bc=bc), res
        except Exception as e:  # transient device/tunnel flakes: retry once
            last_err = e
    raise last_err


def kernel(**inputs):
    outputs, _ = run(inputs)
    return outputs


# revision 11
# speedup vs baseline: 1.1478x; 1.0524x over previous
"""Trainium2 Bass kernel for nn_AttnDecoderLSTM (B=4096, S=512, H=O=128).

Data-parallel over 8 NeuronCores: each core owns 512 batch rows.
All on-chip compute runs in transposed (feature-on-partition) layout;
the host pre-transposes the small per-core activations and weights and
un-transposes the small outputs.
"""

import os
import sys

sys.path.insert(0, "/opt/trn_rl_repo")

import numpy as np

import concourse.bass as bass  # noqa: F401  (AP types)
import concourse.tile as tile
from concourse import bacc, mybir
from concourse.bass_utils import run_bass_kernel_spmd

F32 = mybir.dt.float32
AF = mybir.ActivationFunctionType

B, S, H, O = 4096, 512, 128, 128
N_CORES = 8
BC = B // N_CORES  # batch rows per core (512)
P = 128  # tile size along batch
NCH = S // 128  # s-chunks (4)
NB = int(os.environ.get("K_NB", "4"))  # batch rows per encoder DMA
MATVEC_DT = os.environ.get("K_MVDT", "bf16")  # "f32" | "f32r" | "bf16"
NW = int(os.environ.get("K_NW", "0"))  # HAM-warmer cadence in batch rows (0 = off)
EPOOL = int(os.environ.get("K_EPOOL", "12"))  # encoder tile double-buffer depth
BF16 = mybir.dt.bfloat16

_NC_CACHE: dict = {}


def build_nc(bc: int = BC, nb: int = NB, mv_dt: str = MATVEC_DT, nw: int = NW):
    """Build the per-core Bass program (same program on all cores)."""
    nc = bacc.Bacc("TRN2", target_bir_lowering=False)

    # ---- DRAM I/O (per-core shard shapes) ----
    xT_d = nc.dram_tensor("xT", [O, bc], F32, kind="ExternalInput")
    hT_d = nc.dram_tensor("hT", [H, bc], F32, kind="ExternalInput")
    cT_d = nc.dram_tensor("cT", [H, bc], F32, kind="ExternalInput")
    enc_d = nc.dram_tensor("enc", [bc, S, H], F32, kind="ExternalInput")

    attn_Wt_x_d = nc.dram_tensor("attn_Wt_x", [O, S], F32, kind="ExternalInput")
    attn_Wt_h_d = nc.dram_tensor("attn_Wt_h", [H, S], F32, kind="ExternalInput")
    aux_row_d = nc.dram_tensor("aux_row", [1, S + P], F32, kind="ExternalInput")
    comb_Wt_x_d = nc.dram_tensor("comb_Wt_x", [O, O], F32, kind="ExternalInput")
    comb_Wt_c_d = nc.dram_tensor("comb_Wt_c", [H, O], F32, kind="ExternalInput")
    wpack_d = nc.dram_tensor("wpack", [128, 134], F32, kind="ExternalInput")
    W_ih_t_d = nc.dram_tensor("W_ih_t", [O, 4 * H], F32, kind="ExternalInput")
    W_hh_t_d = nc.dram_tensor("W_hh_t", [H, 4 * H], F32, kind="ExternalInput")
    out_Wt_d = nc.dram_tensor("out_Wt", [H, O], F32, kind="ExternalInput")

    outT_d = nc.dram_tensor("outT", [O, bc], F32, kind="ExternalOutput")
    h1T_d = nc.dram_tensor("h1T", [H, bc], F32, kind="ExternalOutput")
    c1T_d = nc.dram_tensor("c1T", [H, bc], F32, kind="ExternalOutput")
    attn_d = nc.dram_tensor("attn", [bc, S], F32, kind="ExternalOutput")
    hamd_d = nc.dram_tensor("hamd", [128, 4], F32, kind="ExternalOutput")

    ntiles = bc // P

    with tile.TileContext(nc) as tc:
        with (
            tc.tile_pool(name="wpool", bufs=1) as wp,
            tc.tile_pool(name="epool", bufs=EPOOL) as ep,
            tc.tile_pool(name="xpool", bufs=2) as xp,
            tc.tile_pool(name="apool", bufs=2) as ap,
            tc.tile_pool(name="spool", bufs=2) as sp,
            tc.tile_pool(name="ps_logit", bufs=2, space="PSUM") as ps_logit,
            tc.tile_pool(name="ps_ctx", bufs=2, space="PSUM") as ps_ctx,
            tc.tile_pool(name="ps_small", bufs=3, space="PSUM") as ps_small,
            tc.tile_pool(name="ps_ham", bufs=1, space="PSUM") as ps_ham,
        ):
            # ---- constants / weights (loaded once) ----
            # wpack columns: 0:128 identity, 128 comb_b, 129 out_b, 130:134 gbias
            wpack = wp.tile([128, 134], F32)
            nc.scalar.dma_start(wpack[:], wpack_d[:])
            ident = wpack[:, 0:128]
            comb_b = wpack[:, 128:129]
            out_b = wpack[:, 129:130]
            gbias = wpack[:, 130:134]
            aux_row = wp.tile([1, S + P], F32)
            nc.scalar.dma_start(aux_row[:], aux_row_d[:])
            attn_b = aux_row[:, :S]
            ones_row = aux_row[:, S : S + P]
            zz = wp.tile([128, 512], BF16)
            nc.gpsimd.memset(zz[:], 0.0)
            ham_ps = ps_ham.tile([128, 512], F32)

            def wload(dram, shape):
                t = wp.tile(shape, F32, tag=dram.name)
                nc.scalar.dma_start(t[:], dram[:])
                return t

            attn_Wt_x = wload(attn_Wt_x_d, [O, S])
            attn_Wt_h = wload(attn_Wt_h_d, [H, S])
            comb_Wt_x = wload(comb_Wt_x_d, [O, O])
            comb_Wt_c = wload(comb_Wt_c_d, [H, O])
            W_ih_t = wload(W_ih_t_d, [O, 4 * H])
            W_hh_t = wload(W_hh_t_d, [H, 4 * H])
            out_Wt = wload(out_Wt_d, [H, O])

            at_dt = BF16 if mv_dt == "bf16" else F32
            e_dt = BF16 if mv_dt == "bf16" else F32

            xT_all = wp.tile([O, bc], F32)
            hT_all = wp.tile([H, bc], F32)
            cT_all = wp.tile([H, bc], F32)
            nc.scalar.dma_start(xT_all[:], xT_d[:])
            nc.scalar.dma_start(hT_all[:], hT_d[:])
            nc.scalar.dma_start(cT_all[:], cT_d[:])
            h1_all = wp.tile([H, bc], F32)
            c1_all = wp.tile([H, bc], F32)
            out_all = wp.tile([O, bc], F32)

            def stage_a(t):
                """Inputs, attention logits, softmax, transposed attn columns."""
                tsl = slice(t * P, (t + 1) * P)
                xT = xT_all[:, tsl]
                hT = hT_all[:, tsl]
                cT = cT_all[:, tsl]

                logits = ps_logit.tile([P, S], F32)
                nc.tensor.matmul(logits[:], xT, attn_Wt_x[:], start=True, stop=False)
                nc.tensor.matmul(logits[:], hT, attn_Wt_h[:], start=False, stop=False)
                nc.tensor.matmul(
                    logits[:], ones_row, attn_b, start=False, stop=True
                )

                negmax = ap.tile([P, 1], F32, tag="negmax")
                nc.vector.reduce_max(
                    negmax[:], logits[:], axis=mybir.AxisListType.X, negate=True
                )
                A = ap.tile([P, S], F32, tag="A")
                esum = ap.tile([P, 1], F32, tag="esum")
                nc.scalar.activation(
                    A[:], logits[:], AF.Exp, bias=negmax[:], accum_out=esum[:]
                )
                rs = ap.tile([P, 1], F32, tag="rs")
                nc.vector.reciprocal(rs[:], esum[:])
                nc.vector.tensor_scalar_mul(A[:], A[:], rs[:])
                nc.scalar.dma_start(attn_d[tsl, :], A[:])

                ATs = []
                for c in range(NCH):
                    ps = ps_small.tile([128, P], F32, tag="ps_s")
                    nc.tensor.transpose(ps[:], A[:, c::NCH], ident)
                    ATc = ap.tile([128, P], at_dt, tag=f"AT{c}")
                    nc.scalar.copy(ATc[:], ps[:])
                    ATs.append(ATc)
                return dict(xT=xT, hT=hT, cT=cT, ATs=ATs)

            def stage_bcde(t, st):
                tsl = slice(t * P, (t + 1) * P)
                xT, hT, cT, ATs = st["xT"], st["hT"], st["cT"], st["ATs"]

                # ---- stage B: ctx^T accumulation (the heavy stream) ----
                ctxT_ps = ps_ctx.tile([H, P], F32)
                for bb in range(0, P, nb):
                    if nw and bb % nw == 0:
                        nc.tensor.matmul(
                            ham_ps[:], zz[:, :128], zz[:], start=True, stop=True
                        )
                    if t == ntiles - 1 and bb == P - 2 * nb:
                        for _ in range(12):
                            nc.tensor.matmul(
                                ham_ps[:], zz[:, :128], zz[:], start=True, stop=True
                            )
                    et = ep.tile([128, nb, NCH, H], e_dt, tag="E")
                    esrc = enc_d[t * P + bb : t * P + bb + nb, :, :].rearrange(
                        "bb (p j) h -> p bb j h", j=NCH
                    )
                    if mv_dt == "bf16":
                        nc.gpsimd.dma_start(et[:], esrc)
                    else:
                        nc.sync.dma_start(et[:], esrc)
                    for j in range(nb):
                        col = slice(bb + j, bb + j + 1)
                        for c in range(NCH):
                            nc.tensor.matmul(
                                ctxT_ps[:, col],
                                et[:, j, c, :],
                                ATs[c][:, col],
                                start=(c == 0),
                                stop=(c == NCH - 1),
                            )
                ctxT = sp.tile([H, P], F32, tag="ctxT")
                nc.scalar.copy(ctxT[:], ctxT_ps[:])

                # ---- stage C: comb = relu([x, ctx] @ comb_W.T + comb_b) ----
                comb_ps = ps_small.tile([O, P], F32, tag="ps_s")
                nc.tensor.matmul(comb_ps[:], comb_Wt_x[:], xT, start=True, stop=False)
                nc.tensor.matmul(
                    comb_ps[:], comb_Wt_c[:], ctxT[:], start=False, stop=True
                )
                combT = sp.tile([O, P], F32, tag="combT")
                nc.scalar.activation(combT[:], comb_ps[:], AF.Relu, bias=comb_b)

                # ---- stage D: LSTM cell (gates: 0=i, 1=f, 2=g, 3=o) ----
                gact = []
                for g in range(4):
                    gsl = slice(g * H, (g + 1) * H)
                    gps = ps_small.tile([H, P], F32, tag="ps_s")
                    nc.tensor.matmul(
                        gps[:], W_ih_t[:, gsl], combT[:], start=True, stop=False
                    )
                    nc.tensor.matmul(
                        gps[:], W_hh_t[:, gsl], hT, start=False, stop=True
                    )
                    gs = sp.tile([H, P], F32, tag=f"g{g}")
                    if g == 2:
                        nc.scalar.activation(gs[:], gps[:], AF.Tanh, bias=gbias[:, 2:3])
                    else:
                        # sigmoid(x+b) = 0.5*tanh(0.5x+0.5b)+0.5
                        nc.scalar.activation(
                            gs[:], gps[:], AF.Tanh, bias=gbias[:, g : g + 1], scale=0.5
                        )
                        nc.vector.tensor_scalar(
                            gs[:], gs[:], 0.5, 0.5,
                            op0=mybir.AluOpType.mult, op1=mybir.AluOpType.add,
                        )
                    gact.append(gs)

                i_g, f_g, g_g, o_g = gact
                t1 = sp.tile([H, P], F32, tag="t1")
                nc.vector.tensor_mul(t1[:], f_g[:], cT)
                t2 = sp.tile([H, P], F32, tag="t2")
                nc.vector.tensor_mul(t2[:], i_g[:], g_g[:])
                c1T = c1_all[:, tsl]
                nc.vector.tensor_add(c1T, t1[:], t2[:])

                tc1 = sp.tile([H, P], F32, tag="tc1")
                nc.scalar.activation(tc1[:], c1T, AF.Tanh)
                h1T = h1_all[:, tsl]
                nc.vector.tensor_mul(h1T, o_g[:], tc1[:])

                # ---- stage E: out = h1 @ out_W.T + out_b ----
                out_ps = ps_small.tile([O, P], F32, tag="ps_s")
                nc.tensor.matmul(out_ps[:], out_Wt[:], h1T, start=True, stop=True)
                nc.scalar.activation(
                    out_all[:, tsl], out_ps[:], AF.Identity, bias=out_b
                )

            # Software pipeline: stage A of tile t+1 is emitted before the
            # heavy stream of tile t so the PE has the next tile's attention
            # columns ready before its matvecs begin.
            st = stage_a(0)
            for t in range(ntiles):
                nxt = stage_a(t + 1) if t + 1 < ntiles else None
                stage_bcde(t, st)
                st = nxt

            nc.scalar.dma_start(h1T_d[:], h1_all[:])
            nc.scalar.dma_start(c1T_d[:], c1_all[:])
            nc.scalar.dma_start(outT_d[:], out_all[:])

            if True:
                hs = wp.tile([128, 4], F32)
                nc.vector.tensor_copy(hs[:], ham_ps[:, :4])
                nc.scalar.dma_start(hamd_d[:], hs[:])

    nc.compile()
    return nc


def _prep_in_maps(
    input, h, c, encoder_outputs, attn_W, attn_b, comb_W, comb_b,
    W_ih, b_ih, W_hh, b_hh, out_W, out_b, bc: int = BC, n_cores: int = N_CORES,
):
    f32 = np.float32
    x = np.ascontiguousarray(input[:, 0, :], dtype=f32)  # [B, O]
    h0 = np.ascontiguousarray(h[0], dtype=f32)  # [B, H]
    c0 = np.ascontiguousarray(c[0], dtype=f32)  # [B, H]
    enc = np.ascontiguousarray(encoder_outputs, dtype=f32)

    gb = (np.asarray(b_ih, f32) + np.asarray(b_hh, f32)).reshape(4, H)
    gbias = gb.copy()
    for g in (0, 1, 3):
        gbias[g] *= 0.5  # folded into the 0.5*tanh(0.5x+0.5b)+0.5 sigmoid
    wpack = np.zeros((128, 134), f32)
    wpack[:, 0:128] = np.eye(128, dtype=f32)
    wpack[:, 128] = np.asarray(comb_b, f32)
    wpack[:, 129] = np.asarray(out_b, f32)
    wpack[:, 130:134] = gbias.T
    aux_row = np.zeros((1, S + 128), f32)
    aux_row[0, :S] = np.asarray(attn_b, f32)
    aux_row[0, S:] = 1.0
    weights = {
        "attn_Wt_x": np.ascontiguousarray(np.asarray(attn_W, f32).T[:O], f32),
        "attn_Wt_h": np.ascontiguousarray(np.asarray(attn_W, f32).T[O:], f32),
        "aux_row": aux_row,
        "comb_Wt_x": np.ascontiguousarray(np.asarray(comb_W, f32).T[:O], f32),
        "comb_Wt_c": np.ascontiguousarray(np.asarray(comb_W, f32).T[O:], f32),
        "W_ih_t": np.ascontiguousarray(np.asarray(W_ih, f32).T, f32),
        "W_hh_t": np.ascontiguousarray(np.asarray(W_hh, f32).T, f32),
        "wpack": wpack,
        "out_Wt": np.ascontiguousarray(np.asarray(out_W, f32).T, f32),
    }
    in_maps = []
    for cidx in range(n_cores):
        rows = slice(cidx * bc, (cidx + 1) * bc)
        m = dict(weights)
        m["xT"] = np.ascontiguousarray(x[rows].T, f32)
        m["hT"] = np.ascontiguousarray(h0[rows].T, f32)
        m["cT"] = np.ascontiguousarray(c0[rows].T, f32)
        m["enc"] = enc[rows]
        in_maps.append(m)
    return in_maps


def _gather(results, bc: int = BC):
    out = np.concatenate([r["outT"].T for r in results], axis=0)
    h1 = np.concatenate([r["h1T"].T for r in results], axis=0)
    c1 = np.concatenate([r["c1T"].T for r in results], axis=0)
    attn = np.concatenate([r["attn"] for r in results], axis=0)
    return (
        np.ascontiguousarray(out, np.float32),
        np.ascontiguousarray(h1, np.float32)[None],
        np.ascontiguousarray(c1, np.float32)[None],
        np.ascontiguousarray(attn, np.float32),
    )


def _install_ntff_hook():
    """The image's antenv lacks axon_hooks; provide it and register the
    ctypes NTFF profiling hook so trace=True yields exec_time_ns."""
    import sys
    import types

    if "antenv.axon_hooks" in sys.modules:
        return
    import antenv

    mod = types.ModuleType("antenv.axon_hooks")
    _holder = {"hook": None}
    mod.set_axon_ntff_profile_hook = lambda h: _holder.__setitem__("hook", h)
    mod.get_axon_ntff_profile_hook = lambda: _holder["hook"]
    sys.modules["antenv.axon_hooks"] = mod
    antenv.axon_hooks = mod
    try:
        from trn_agent_boot.trn_boot import _ntff_profile_via_ctypes

        hook = _ntff_profile_via_ctypes("/opt/axon/libaxon_pjrt.so")
        if hook is not None:
            mod.set_axon_ntff_profile_hook(hook)
    except Exception as e:  # degrade: tracing skipped, run still works
        print(f"ntff hook install failed: {e}")


def run(inputs: dict, trace: bool = False, bc: int = BC, n_cores: int = N_CORES):
    if trace:
        _install_ntff_hook()
    key = (bc, n_cores)
    if key not in _NC_CACHE:
        _NC_CACHE[key] = build_nc(bc=bc)
    nc = _NC_CACHE[key]
    in_maps = _prep_in_maps(**inputs, bc=bc, n_cores=n_cores)
    last_err = None
    for attempt in range(2):
        try:
            res = run_bass_kernel_spmd(
                nc, in_maps, list(range(n_cores)), trace=trace,
                tmpdir=os.environ.get("BASS_TRACE_DIR"),
            )
            return _gather(res.results, bc=bc), res
        except Exception as e:  # transient device/tunnel flakes: retry once
            last_err = e
    raise last_err


def kernel(**inputs):
    outputs, _ = run(inputs)
    return outputs
